# revision 1
# baseline (speedup 1.0000x reference)
# DenseGATv2Conv Trainium2 kernel.
#
# Math (per batch b):
#   xl = x @ W_l + b_l ; xr = x @ W_r + b_r            [N, H*C]
#   alpha[i,j,h] = sum_c att[h,c] * leaky_relu(xl[j,hc] + xr[i,hc], 0.2)
#   S = softmax_j(alpha masked by adj(+self loops))
#   out[i,hc] = sum_j S[i,j,h] * xr[j,hc] + bias
#
# Key identity used on device:
#   leaky_relu(z) = 0.2*z + 0.8*relu(z)
#   alpha[i,j,h] = 0.2*sl[j,h] + 0.2*sr[i,h] + 0.8*sum_c att[h,c]*relu(xl[j,hc]+xr[i,hc])
# where sl = xl @ att_blk, sr = xr @ att_blk are rank-1 in the (i,j) plane.
# In softmax over j the exp(0.2*sr[i,h]) factor cancels; exp(0.2*sl[j,h]) is
# folded multiplicatively into the aggregation operand. So the only O(N^2*HC)
# work is ONE fused elementwise op per destination row pair: relu(xl[j]+xr[i])
# followed by a tensor-engine contraction with a block-diagonal att matrix.
#
# Sharding: 8 cores = (batch b in 0..1) x (4 blocks of 256 destination rows).
# Each core gets full x[b], its 256-row slice of adj (with self-loops set
# host-side), and returns its [256, 64] slice of the output.

import numpy as np

B, N, F, H, C = 2, 1024, 128, 4, 16
HC = H * C
NCORES = 8
NI = 256          # destination rows per core
NPAIR = NI // 2   # 128 pairs of destination rows
NSUP = 8          # supers of 16 pairs (32 dest rows) each
NEG = 0.2

_CACHE = {}
LAST_RESULTS = None


GEN_ACT_MOD = 4


def _build_program(debug=False):
    import concourse.bass as bass
    import concourse.mybir as mybir
    import concourse.tile as tile
    from concourse import bacc

    f32 = mybir.dt.float32
    f32r = mybir.dt.float32r
    f16 = mybir.dt.float16
    Alu = mybir.AluOpType
    Act = mybir.ActivationFunctionType

    nc = bacc.Bacc(
        "TRN2",
        target_bir_lowering=False,
        debug=False,
        enable_asserts=False,
        num_devices=NCORES,
    )

    # ---- DRAM I/O ----
    xb = nc.dram_tensor("xb", [N, F], f32, kind="ExternalInput").ap()
    xis = nc.dram_tensor("xis", [NI, F], f32, kind="ExternalInput").ap()
    adjs = nc.dram_tensor("adjs", [NI, N], f16, kind="ExternalInput").ap()
    wl = nc.dram_tensor("wl", [F, HC], f32, kind="ExternalInput").ap()
    wr = nc.dram_tensor("wr", [F, HC], f32, kind="ExternalInput").ap()
    blp = nc.dram_tensor("blp", [HC, 1], f32, kind="ExternalInput").ap()
    brp = nc.dram_tensor("brp", [HC, 1], f32, kind="ExternalInput").ap()
    att2p = nc.dram_tensor("att2p", [F, 32], f32, kind="ExternalInput").ap()
    attb = nc.dram_tensor("attb", [HC, H], f32, kind="ExternalInput").ap()
    ident = nc.dram_tensor("ident", [128, 128], f32, kind="ExternalInput").ap()
    biasb = nc.dram_tensor("biasb", [128, HC], f32, kind="ExternalInput").ap()
    out = nc.dram_tensor("out", [NI, HC], f32, kind="ExternalOutput").ap()
    dbg_t = {}
    if debug:
        f16_ = mybir.dt.float16
        for nm, shp, dt_ in [("d_xl2T", [128, N], f32), ("d_xrp", [128, NPAIR], f32),
                             ("d_eslT", [H, N], f32), ("d_xrmod", [128, 544], f32),
                             ("d_adjT", [128, 2 * N], f16_), ("d_rp0", [128, N], f16_),
                             ("d_ssb0", [128, N], f16_), ("d_scomp0", [128, N], f16_),
                             ("d_stt0", [128, 4096], f16_)]:
            dbg_t[nm] = nc.dram_tensor(nm, shp, dt_, kind="ExternalOutput").ap()

    with tile.TileContext(nc) as tc:
        _body(tc, nc, mybir, bass, f32, f32r, Alu, Act,
              f16, xb, xis, adjs, wl, wr, blp, brp, att2p, attb, ident, biasb, out,
              dbg_t)

    nc.compile()
    return nc


def _body(tc, nc, mybir, bass, f32, f32r, Alu, Act,
          f16, xb, xis, adjs, wl, wr, blp, brp, att2p, attb, ident, biasb, out,
          dbg_t=None):
    from contextlib import ExitStack
    ctx = ExitStack()
    with ctx:
        consts = ctx.enter_context(tc.tile_pool(name="consts", bufs=1))
        work = ctx.enter_context(tc.tile_pool(name="work", bufs=1))
        rp_pool = ctx.enter_context(tc.tile_pool(name="rp", bufs=8))
        ssb_pool = ctx.enter_context(tc.tile_pool(name="ssb", bufs=3))
        scomp_pool = ctx.enter_context(tc.tile_pool(name="scomp", bufs=2))
        outp = ctx.enter_context(tc.tile_pool(name="outp", bufs=2))
        psg = ctx.enter_context(tc.tile_pool(name="psg", bufs=2, space="PSUM"))
        pst = ctx.enter_context(tc.tile_pool(name="pst", bufs=4, space="PSUM"))

        dma = nc.sync.dma_start

        # ---------- load constants ----------
        wl_t = consts.tile([F, HC], f32, tag="wl")
        wr_t = consts.tile([F, HC], f32, tag="wr")
        blp_t = consts.tile([HC, 1], f32, tag="blp")
        brp_t = consts.tile([HC, 1], f32, tag="brp")
        att2p_t = consts.tile([F, 32], f32, tag="att2p")
        att2p_r = consts.tile([F, 32], f16, tag="att2pr")
        attb_t = consts.tile([HC, H], f32, tag="attb")
        wl_r = consts.tile([F, HC], f32r, tag="wlr")
        wr_r = consts.tile([F, HC], f32r, tag="wrr")
        attb_r = consts.tile([HC, H], f32r, tag="attbr")
        id_t = consts.tile([128, 128], f32, tag="ident")
        biasb_t = consts.tile([128, HC], f32, tag="biasb")
        dma(wl_t[:], wl)
        dma(wr_t[:], wr)
        dma(blp_t[:], blp)
        dma(brp_t[:], brp)
        dma(att2p_t[:], att2p)
        nc.vector.tensor_copy(att2p_r[:], att2p_t[:])
        nc.vector.tensor_copy(wl_r[:], wl_t[:])
        nc.vector.tensor_copy(wr_r[:], wr_t[:])
        dma(attb_t[:], attb)
        nc.vector.tensor_copy(attb_r[:], attb_t[:])
        dma(id_t[:], ident)
        dma(biasb_t[:], biasb)

        # ---------- load x, adj ----------
        xin = consts.tile([128, 8 * F], f32, tag="xin")       # x[b] tiles, [node128][f]
        for k in range(8):
            dma(xin[:, k * F:(k + 1) * F], xb[k * 128:(k + 1) * 128, :])
        xis_t = consts.tile([128, 2 * F], f32, tag="xis")
        for k in range(2):
            dma(xis_t[:, k * F:(k + 1) * F], xis[k * 128:(k + 1) * 128, :])
        adjm = consts.tile([128, 2 * N], f16, tag="adjm")     # [i128][ib*N + j]
        for ib in range(2):
            dma(adjm[:, ib * N:(ib + 1) * N], adjs[ib * 128:(ib + 1) * 128, :])

        # ---------- xT via PE transpose ----------
        xT = consts.tile([F, N], f32r, tag="xT")              # [f, node]
        for k in range(8):
            pt = pst.tile([128, 128], f32, tag="pt")
            nc.tensor.transpose(pt[:], xin[:, k * F:(k + 1) * F], id_t[:])
            nc.vector.tensor_copy(xT[:, k * 128:(k + 1) * 128], pt[:])
        xisT = consts.tile([F, NI], f32r, tag="xisT")
        for k in range(2):
            pt = pst.tile([128, 128], f32, tag="pt")
            nc.tensor.transpose(pt[:], xis_t[:, k * F:(k + 1) * F], id_t[:])
            nc.vector.tensor_copy(xisT[:, k * 128:(k + 1) * 128], pt[:])

        # ---------- projections:  xl2T = (x@W_l + b_l)^T stacked twice ----------
        xl2T = consts.tile([128, N], f16, tag="xl2T")         # fp16: rows 0:64 == 64:128
        xlT32 = consts.tile([HC, N], f32r, tag="xlT32")       # f32r copy for slT matmul
        xrT = consts.tile([HC, N], f32, tag="xrT")            # (x@W_r+b_r)^T, all nodes
        xrsT = consts.tile([HC, NI], f32, tag="xrsT")         # same, dest-row slice
        pj = psg.tile([HC, N], f32, tag="g")
        for half in range(2):
            s = slice(half * 512, (half + 1) * 512)
            nc.tensor.matmul(pj[:, s], wl_r[:], xT[:, s],
                             start=True, stop=True)
        nc.scalar.activation(xl2T[0:HC, :], pj[:], Act.Identity,
                             bias=blp_t[:, 0:1], scale=1.0)
        nc.scalar.activation(xl2T[HC:128, :], pj[:], Act.Identity,
                             bias=blp_t[:, 0:1], scale=1.0)
        nc.scalar.activation(xlT32[:], pj[:], Act.Identity,
                             bias=blp_t[:, 0:1], scale=1.0)
        pj2 = psg.tile([HC, N], f32, tag="g")
        for half in range(2):
            s = slice(half * 512, (half + 1) * 512)
            nc.tensor.matmul(pj2[:, s], wr_r[:], xT[:, s],
                             start=True, stop=True)
        nc.scalar.activation(xrT[:], pj2[:], Act.Identity,
                             bias=brp_t[:, 0:1], scale=1.0)
        pj3 = psg.tile([HC, NI], f32, tag="g")
        nc.tensor.matmul(pj3[:], wr_r[:], xisT[:],
                         start=True, stop=True)
        nc.scalar.activation(xrsT[:], pj3[:], Act.Identity,
                             bias=brp_t[:, 0:1], scale=1.0)

        # ---------- xrp: per-pair bias columns [xr[i0+2p] ; xr[i0+2p+1]] ----------
        xrp = consts.tile([128, NPAIR], f32, tag="xrp")
        ev = xrsT[:].rearrange("p (a two) -> p a two", two=2)
        nc.vector.tensor_copy(xrp[0:HC, :], ev[:, :, 0])
        nc.vector.tensor_copy(xrp[HC:128, :], ev[:, :, 1])

        # Deferred builds (emitted inside the main loop so the first contract
        # matmuls are not delayed): adjT before super-0 transposes, xr_mod
        # after super 0.
        xr_mod = consts.tile([128, 8 * 68], f16, tag="xrmod")
        adjT = consts.tile([128, 2 * N], f16, tag="adjT")
        id16 = consts.tile([128, 128], f16, tag="id16")
        nc.vector.tensor_copy(id16[:], id_t[:])

        def build_adjT(ib2):
            if True:
                for k in range(8):
                    pt = pst.tile([128, 128], f16, tag="pt", name="pt")
                    nc.tensor.transpose(pt[:], adjm[:, ib2 * N + k * 128: ib2 * N + (k + 1) * 128],
                                        id16[:])
                    nc.vector.tensor_copy(adjT[:, k * 256 + ib2 * 128: k * 256 + (ib2 + 1) * 128],
                                          pt[:])

        def build_xr_mod():
            # slT[h, j] = sum_hc att_blk[hc,h]*xl[hc,j];  esl = exp(0.2*sl)
            psl = psg.tile([H, N], f32, tag="g", name="psl")
            for half in range(2):
                s = slice(half * 512, (half + 1) * 512)
                nc.tensor.matmul(psl[:, s], attb_r[:],
                                 xlT32[:, s], start=True, stop=True)
            eslT = work.tile([H, N], f32, tag="eslT", name="eslT")
            nc.scalar.activation(eslT[:], psl[:], Act.Exp, scale=0.2)
            esl_nat = work.tile([128, 8 * H], f32, tag="eslnat", name="esl_nat")
            for k in range(8):
                pt = pst.tile([128, 128], f32, tag="pt", name="pt")
                nc.tensor.transpose(pt[:, 0:H], eslT[:, k * 128:(k + 1) * 128],
                                    id_t[0:H, 0:H])
                nc.vector.tensor_copy(esl_nat[:, k * H:(k + 1) * H], pt[:, 0:H])
                zcols = xr_mod[:].rearrange("p (k h c) -> p k h c", k=8, h=H)[:, k, :, 16]
                nc.vector.tensor_copy(zcols, pt[:, 0:H])
                pt2 = pst.tile([128, 128], f32, tag="pt", name="pt2")
                nc.tensor.transpose(pt2[:, 0:HC], xrT[:, k * 128:(k + 1) * 128],
                                    id_t[0:HC, 0:HC])
                xcols = xr_mod[:].rearrange("p (k h c) -> p k h c", k=8, h=H)[:, k, :, 0:16]
                srcx = pt2[:, 0:HC].rearrange("p (h c) -> p h c", h=H)
                rep = esl_nat[:, k * H:(k + 1) * H].rearrange("p (h one) -> p h one", one=1).broadcast_to([128, H, 16])
                nc.vector.tensor_tensor(xcols, srcx, rep, Alu.mult)
            if dbg_t:
                dma(dbg_t["d_eslT"], eslT[:])

        if dbg_t:
            dma(dbg_t["d_xl2T"], xl2T[:])
            dma(dbg_t["d_xrp"], xrp[:])

        # ---------- main streaming loop ----------
        st_t = [consts.tile([128, 8 * 512], f16, tag=f"stt{ib}", name=f"stt{ib}") for ib in range(2)]

        def aggregate(ib):
            out_f = outp.tile([128, HC], f32, tag="outf", name="outf")
            out_f2 = outp.tile([128, HC], f32, tag="outf2", name="outf2")
            for h in range(H):
                agg = psg.tile([128, 17], f32, tag="g", name="agg")
                for k in range(8):
                    lhs = st_t[ib][:].rearrange("p (k s a h) -> p k s a h",
                                                k=8, s=4, h=H)[:, k, :, :, h]
                    rhs = xr_mod[:, k * 68 + h * 17: k * 68 + (h + 1) * 17]
                    nc.tensor.matmul(agg[:], lhs, rhs,
                                     start=(k == 0), stop=(k == 7))
                rz = work.tile([128, 1], f32, tag="rz", name="rz")
                nc.vector.reciprocal(rz[:], agg[:, 16:17])
                nc.vector.tensor_scalar(out_f[:, h * 16:(h + 1) * 16],
                                        agg[:, 0:16], rz[:, 0:1], None, Alu.mult)
            nc.vector.tensor_add(out_f2[:], out_f[:], biasb_t[:])
            dma(out[ib * 128:(ib + 1) * 128, :], out_f2[:])

        for sup in range(NSUP):
            ib, s4 = sup // 4, sup % 4
            if sup == 1:
                build_xr_mod()
            scomp = scomp_pool.tile([128, N], f16, tag="scomp")
            for g in range(4):
                gps = psg.tile([128, N], f32, tag="g")
                for q4 in range(4):
                    p = sup * 16 + g * 4 + q4
                    rp = rp_pool.tile([128, N], f16, tag="rp")
                    if q4 >= GEN_ACT_MOD:
                        nc.scalar.activation(rp[:], xl2T[:], Act.Relu,
                                             bias=xrp[:, p:p + 1], scale=1.0)
                    else:
                        nc.vector.tensor_scalar(rp[:], xl2T[:],
                                                xrp[:, p:p + 1],
                                                0.0, Alu.add, Alu.max)
                    if dbg_t and p == 0:
                        dma(dbg_t["d_rp0"], rp[:])
                    for half in range(2):
                        s = slice(half * 512, (half + 1) * 512)
                        nc.tensor.matmul(gps[32 * q4:32 * q4 + 32, s],
                                         att2p_r[:],
                                         rp[:, s],
                                         start=True, stop=True,
                                         tile_position=(0, 32 * q4))
                ssb = ssb_pool.tile([128, N], f16, tag="ssb")
                nc.scalar.activation(ssb[:], gps[:], Act.Exp)
                if dbg_t and sup == 0 and g == 0:
                    dma(dbg_t["d_ssb0"], ssb[:])
                for c4 in range(4):
                    dma(scomp[g * 32 + c4 * 8: g * 32 + (c4 + 1) * 8, :],
                        ssb[32 * c4:32 * c4 + 8, :])
            if dbg_t and sup == 0:
                dma(dbg_t["d_scomp0"], scomp[:])
            if sup == 0:
                build_adjT(0)
            elif sup == 2:
                build_adjT(1)
            for k in range(8):
                pt = pst.tile([128, 128], f16, tag="pt", name="pt")
                nc.tensor.transpose(pt[:], scomp[:, k * 128:(k + 1) * 128], id16[:])
                msk = adjT[:, k * 256 + ib * 128 + s4 * 32:
                           k * 256 + ib * 128 + s4 * 32 + 32]
                mskr = msk.rearrange("p (a one) -> p a one", one=1).broadcast_to([128, 32, H])
                dstv = st_t[ib][:, k * 512 + s4 * 128: k * 512 + (s4 + 1) * 128]
                dstv = dstv.rearrange("p (a h) -> p a h", h=H)
                ptv = pt[:].rearrange("p (a h) -> p a h", h=H)
                nc.vector.tensor_tensor(dstv, ptv, mskr, Alu.mult)


        if dbg_t:
            dma(dbg_t["d_stt0"], st_t[0][:])
            dma(dbg_t["d_xrmod"], xr_mod[:])
            dma(dbg_t["d_adjT"], adjT[:])
        aggregate(0)
        aggregate(1)


def _get_program():
    if "nc" not in _CACHE:
        _CACHE["nc"] = _build_program()
    return _CACHE["nc"]


def kernel(x, adj, W_l, b_l, W_r, b_r, att, bias):
    global LAST_RESULTS
    from concourse.bass_utils import run_bass_kernel_spmd

    x = np.ascontiguousarray(np.asarray(x, dtype=np.float32))
    adj = np.ascontiguousarray(np.asarray(adj, dtype=np.float32))
    W_l = np.asarray(W_l, dtype=np.float32)
    b_l = np.asarray(b_l, dtype=np.float32)
    W_r = np.asarray(W_r, dtype=np.float32)
    b_r = np.asarray(b_r, dtype=np.float32)
    att = np.asarray(att, dtype=np.float32)
    bias = np.asarray(bias, dtype=np.float32)

    # host-side constant prep
    att2p = np.zeros((F, 32), np.float32)        # [0.8 * att2 | 0]
    for d in range(2):
        for h in range(H):
            att2p[d * HC + h * C:(d * HC + (h + 1) * C), d * H + h] = 0.8 * att[h]
    attb = np.zeros((HC, H), np.float32)         # att_blk
    for h in range(H):
        attb[h * C:(h + 1) * C, h] = att[h]
    identity = np.eye(128, dtype=np.float32)
    biasb = np.broadcast_to(bias, (128, HC)).copy()
    blp = b_l.reshape(HC, 1).copy()
    brp = b_r.reshape(HC, 1).copy()

    in_maps = []
    for core in range(NCORES):
        b, blk = core // 4, core % 4
        i0 = blk * NI
        adjs = adj[b, i0:i0 + NI, :].copy()
        adjs[np.arange(NI), i0 + np.arange(NI)] = 1.0   # self loops
        adjs = adjs.astype(np.float16)
        in_maps.append({
            "xb": x[b], "xis": x[b, i0:i0 + NI].copy(), "adjs": adjs,
            "wl": W_l, "wr": W_r, "blp": blp, "brp": brp,
            "att2p": att2p, "attb": attb, "ident": identity, "biasb": biasb,
        })

    nc = _get_program()
    res = run_bass_kernel_spmd(nc, in_maps, core_ids=list(range(NCORES)))
    LAST_RESULTS = res
    outp = np.zeros((B, N, HC), np.float32)
    for core in range(NCORES):
        b, blk = core // 4, core % 4
        outp[b, blk * NI:(blk + 1) * NI, :] = res.results[core]["out"]
    return outp



# revision 6
# speedup vs baseline: 1.2067x; 1.2067x over previous
# DenseGATv2Conv Trainium2 kernel (v2).
#
# Math (per batch b):
#   xl = x @ W_l + b_l ; xr = x @ W_r + b_r            [N, H*C]
#   alpha[i,j,h] = sum_c att[h,c] * leaky_relu(xl[j,hc] + xr[i,hc], 0.2)
#   S = softmax_j(alpha masked by adj(+self loops))
#   out[i,hc] = sum_j S[i,j,h] * xr[j,hc] + bias
#
# Identity used on device:
#   leaky_relu(z) = 0.2*z + 0.8*relu(z)
#   alpha[i,j,h] = 0.2*sl[j,h] + 0.2*sr[i,h] + 0.8*sum_c att[h,c]*relu(...)
# exp(0.2*sr[i,h]) cancels in softmax; exp(0.2*sl[j,h]) is folded into the
# aggregation operand (xr_mod).  Per pair of destination rows (i0,i1) the
# relu tensor rp[128=2x64 hc, N j] feeds a PE contraction with att.
#
# v2 structure (vs baseline):
#  - scores for the last NSUP8 supers use fp8e4 + DoubleRow matmuls packing
#    TWO pairs per matmul (0.5 cy/row): 4x tensor-engine rate, and halves
#    the Act-engine exp count (16 useful rows per 32-row PSUM block).
#    att quantization error is cancelled by pre-scaling xl/xr rows with
#    r = att/q8(att) (relu is positively homogeneous).
#  - partition compaction of the sparse score blocks is fused into the PE
#    transpose by using a selection matrix instead of the identity
#    (out = ssb^T @ sel), eliminating all 128 SBUF->SBUF gather DMAs of the
#    baseline (HWDGE was 81% busy).
#  - per-super transposes write one PSUM tile (pt_all); masking by adjT
#    runs on the otherwise-idle Pool engine and writes st directly.
#  - relu ops are spread across DVE/Act/Pool to balance engine busy time.
#
# Sharding: 8 cores = (batch b in 0..1) x (4 blocks of 256 destination rows).

import numpy as np

B, N, F, H, C = 2, 1024, 128, 4, 16
HC = H * C
NCORES = 8
NI = 256          # destination rows per core
NPAIR = NI // 2   # 128 pairs of destination rows
NSUP = 8          # supers of 16 pairs (32 dest rows) each
NSUP8 = 0         # how many (trailing) supers use the fp8 DoubleRow path

# engine assignment for the fp8 relu ops (cycled): d=DVE, a=Act, p=Pool
RP8_ENGINES = "ddapdap"
# engine assignment for f16 relu ops
RP16_ENGINES = "d"
# mask op engine: "p" = Pool (needs a PSUM->SBUF bounce copy), "d" = DVE direct
MASK_ENGINE = "p"
# engine for the pt_all PSUM->SBUF bounce copy: "d"=DVE, "a"=Act
PTCOPY_ENGINE = "d"

_CACHE = {}
LAST_RESULTS = None


def _q8(v):
    """Round to float8e4m3 grid (RNE), pure numpy. No inf/nan handling."""
    v = np.clip(np.asarray(v, np.float64), -448.0, 448.0)
    m, e = np.frexp(v)  # v = m * 2**e, |m| in [0.5, 1)
    normal = np.abs(v) >= 2.0 ** -6
    # normal: round to 4-bit significand at scale 2**(e-4)
    qn = np.round(v * 2.0 ** (4 - e)) * 2.0 ** (e - 4.0)
    # subnormal: step 2**-9
    qs = np.round(v * 2.0 ** 9) * 2.0 ** -9
    return np.where(normal, qn, qs).astype(np.float32)


def _build_program(debug=False):
    import concourse.bass as bass
    import concourse.mybir as mybir
    import concourse.tile as tile
    from concourse import bacc

    f32 = mybir.dt.float32
    f32r = mybir.dt.float32r
    f16 = mybir.dt.float16
    f8e4 = mybir.dt.float8e4
    Alu = mybir.AluOpType
    Act = mybir.ActivationFunctionType

    nc = bacc.Bacc(
        "TRN2",
        target_bir_lowering=False,
        debug=False,
        enable_asserts=False,
        num_devices=NCORES,
    )

    # ---- DRAM I/O ----
    xb = nc.dram_tensor("xb", [N, F], f32, kind="ExternalInput").ap()
    xis = nc.dram_tensor("xis", [NI, F], f32, kind="ExternalInput").ap()
    adjs = nc.dram_tensor("adjs", [NI, N], f16, kind="ExternalInput").ap()
    wl = nc.dram_tensor("wl", [F, HC], f32, kind="ExternalInput").ap()
    wr = nc.dram_tensor("wr", [F, HC], f32, kind="ExternalInput").ap()
    blp = nc.dram_tensor("blp", [HC, 1], f32, kind="ExternalInput").ap()
    brp = nc.dram_tensor("brp", [HC, 1], f32, kind="ExternalInput").ap()
    att2p = nc.dram_tensor("att2p", [F, 32], f32, kind="ExternalInput").ap()
    attb = nc.dram_tensor("attb", [HC, H], f32, kind="ExternalInput").ap()
    ident = nc.dram_tensor("ident", [128, 128], f32, kind="ExternalInput").ap()
    biasb = nc.dram_tensor("biasb", [128, HC], f32, kind="ExternalInput").ap()
    att8w = nc.dram_tensor("att8w", [128, 64], f32, kind="ExternalInput").ap()
    r2 = nc.dram_tensor("r2", [128, 1], f32, kind="ExternalInput").ap()
    selw = nc.dram_tensor("selw", [128, 96], f32, kind="ExternalInput").ap()
    out = nc.dram_tensor("out", [NI, HC], f32, kind="ExternalOutput").ap()

    with tile.TileContext(nc) as tc:
        _body(tc, nc, mybir, bass, f32, f32r, f16, f8e4, Alu, Act,
              xb, xis, adjs, wl, wr, blp, brp, att2p, attb, ident, biasb,
              att8w, r2, selw, out)

    nc.compile()
    return nc


def _body(tc, nc, mybir, bass, f32, f32r, f16, f8e4, Alu, Act,
          xb, xis, adjs, wl, wr, blp, brp, att2p, attb, ident, biasb,
          att8w, r2, selw, out):
    from contextlib import ExitStack
    ctx = ExitStack()
    with ctx:
        consts = ctx.enter_context(tc.tile_pool(name="consts", bufs=1))
        work = ctx.enter_context(tc.tile_pool(name="work", bufs=1))
        rp_pool = ctx.enter_context(tc.tile_pool(name="rp", bufs=6))
        rp8_pool = ctx.enter_context(tc.tile_pool(name="rp8", bufs=4))
        ssb_pool = ctx.enter_context(tc.tile_pool(name="ssb", bufs=3))
        outp = ctx.enter_context(tc.tile_pool(name="outp", bufs=2))
        psg = ctx.enter_context(tc.tile_pool(name="psg", bufs=3, space="PSUM"))
        pta = ctx.enter_context(tc.tile_pool(name="pta", bufs=2, space="PSUM"))

        dma = nc.sync.dma_start

        def rp_issue(eng, dst, src, bias_col):
            if eng == "d":
                nc.vector.tensor_scalar(dst, src, bias_col, 0.0, Alu.add, Alu.max)
            elif eng == "a":
                nc.scalar.activation(dst, src, Act.Relu, bias=bias_col, scale=1.0)
            else:
                nc.gpsimd.tensor_scalar(dst, src, bias_col, 0.0, Alu.add, Alu.max)

        # ---------- load constants ----------
        wl_t = consts.tile([F, HC], f32, tag="wl")
        wr_t = consts.tile([F, HC], f32, tag="wr")
        blp_t = consts.tile([HC, 1], f32, tag="blp")
        brp_t = consts.tile([HC, 1], f32, tag="brp")
        att2p_t = consts.tile([F, 32], f32, tag="att2p")
        att2p_r = consts.tile([F, 32], f16, tag="att2pr")
        attb_t = consts.tile([HC, H], f32, tag="attb")
        wl_r = consts.tile([F, HC], f32r, tag="wlr")
        wr_r = consts.tile([F, HC], f32r, tag="wrr")
        attb_r = consts.tile([HC, H], f32r, tag="attbr")
        id_t = consts.tile([128, 128], f32, tag="ident")
        biasb_t = consts.tile([128, HC], f32, tag="biasb")
        att8w_t = consts.tile([128, 64], f32, tag="att8w")
        att8 = consts.tile([128, 2, 32], f8e4, tag="att8")
        r2_t = consts.tile([128, 1], f32, tag="r2")
        selw_t = consts.tile([128, 96], f32, tag="selw")
        sel32 = consts.tile([128, 32], f16, tag="sel32")
        sel64 = consts.tile([128, 64], f16, tag="sel64")
        dma(wl_t[:], wl)
        dma(wr_t[:], wr)
        dma(blp_t[:], blp)
        dma(brp_t[:], brp)
        dma(att2p_t[:], att2p)
        nc.vector.tensor_copy(att2p_r[:], att2p_t[:])
        nc.vector.tensor_copy(wl_r[:], wl_t[:])
        nc.vector.tensor_copy(wr_r[:], wr_t[:])
        dma(attb_t[:], attb)
        nc.vector.tensor_copy(attb_r[:], attb_t[:])
        dma(id_t[:], ident)
        dma(biasb_t[:], biasb)
        dma(att8w_t[:], att8w)
        nc.gpsimd.tensor_copy(att8[:].rearrange("p t m -> p (t m)"), att8w_t[:])
        dma(r2_t[:], r2)
        dma(selw_t[:], selw)
        nc.gpsimd.tensor_copy(sel32[:], selw_t[:, 0:32])
        nc.gpsimd.tensor_copy(sel64[:], selw_t[:, 32:96])
        id16 = consts.tile([128, 128], f16, tag="id16")
        nc.vector.tensor_copy(id16[:], id_t[:])

        # ---------- load x, adj ----------
        xin = consts.tile([128, 8 * F], f32, tag="xin")       # x[b] tiles
        for k in range(8):
            dma(xin[:, k * F:(k + 1) * F], xb[k * 128:(k + 1) * 128, :])
        xis_t = consts.tile([128, 2 * F], f32, tag="xis")
        for k in range(2):
            dma(xis_t[:, k * F:(k + 1) * F], xis[k * 128:(k + 1) * 128, :])
        adjm = consts.tile([128, 2 * N], f16, tag="adjm")     # [i128][ib*N + j]
        for ib in range(2):
            dma(adjm[:, ib * N:(ib + 1) * N], adjs[ib * 128:(ib + 1) * 128, :])

        # ---------- xT via PE transpose (batched through one psum tile) ----------
        xT = consts.tile([F, N], f32r, tag="xT")              # [f, node]
        gsc = psg.tile([128, N], f32, tag="g", name="gsc_xT")
        for k in range(8):
            nc.tensor.transpose(gsc[:, k * 128:(k + 1) * 128],
                                xin[:, k * F:(k + 1) * F], id_t[:])
        nc.vector.tensor_copy(xT[:], gsc[:])
        xisT = consts.tile([F, NI], f32r, tag="xisT")
        gsc2 = psg.tile([128, N], f32, tag="g", name="gsc_xisT")
        for k in range(2):
            nc.tensor.transpose(gsc2[:, k * 128:(k + 1) * 128],
                                xis_t[:, k * F:(k + 1) * F], id_t[:])
        nc.vector.tensor_copy(xisT[:], gsc2[:, 0:NI])

        # ---------- projections ----------
        xl2T = consts.tile([128, N], f16, tag="xl2T")         # rows 0:64 == 64:128
        xlT32 = consts.tile([HC, N], f32r, tag="xlT32")
        xrT = consts.tile([HC, N], f32, tag="xrT")
        xrsT = consts.tile([HC, NI], f32, tag="xrsT")
        pj = psg.tile([HC, N], f32, tag="g", name="pj")
        for half in range(2):
            s = slice(half * 512, (half + 1) * 512)
            nc.tensor.matmul(pj[:, s], wl_r[:], xT[:, s], start=True, stop=True)
        nc.scalar.activation(xl2T[0:HC, :], pj[:], Act.Identity,
                             bias=blp_t[:, 0:1], scale=1.0)
        nc.scalar.activation(xl2T[HC:128, :], pj[:], Act.Identity,
                             bias=blp_t[:, 0:1], scale=1.0)
        nc.scalar.activation(xlT32[:], pj[:], Act.Identity,
                             bias=blp_t[:, 0:1], scale=1.0)
        pj2 = psg.tile([HC, N], f32, tag="g", name="pj2")
        for half in range(2):
            s = slice(half * 512, (half + 1) * 512)
            nc.tensor.matmul(pj2[:, s], wr_r[:], xT[:, s], start=True, stop=True)
        nc.scalar.activation(xrT[:], pj2[:], Act.Identity,
                             bias=brp_t[:, 0:1], scale=1.0)
        pj3 = psg.tile([HC, NI], f32, tag="g", name="pj3")
        nc.tensor.matmul(pj3[:], wr_r[:], xisT[:], start=True, stop=True)
        nc.scalar.activation(xrsT[:], pj3[:], Act.Identity,
                             bias=brp_t[:, 0:1], scale=1.0)

        # ---------- xrp: per-pair bias columns [xr[i0+2p] ; xr[i0+2p+1]] ----------
        xrp = consts.tile([128, NPAIR], f32, tag="xrp")
        ev = xrsT[:].rearrange("p (a two) -> p a two", two=2)
        nc.vector.tensor_copy(xrp[0:HC, :], ev[:, :, 0])
        nc.vector.tensor_copy(xrp[HC:128, :], ev[:, :, 1])

        # fp8 path: r-scaled copies (cancels att quantization error)
        if NSUP8 > 0:
            xl2T8 = consts.tile([128, N], f16, tag="xl2T8")
            xrp8 = consts.tile([128, NPAIR], f32, tag="xrp8")
            nc.vector.tensor_scalar(xl2T8[:], xl2T[:], r2_t[:, 0:1], None, Alu.mult)
            nc.vector.tensor_scalar(xrp8[:], xrp[:], r2_t[:, 0:1], None, Alu.mult)

        # ---------- adjT via PE transposes batched through pta tiles ----------
        adjT = consts.tile([128, 2 * N], f16, tag="adjT")
        for ib in range(2):
            ptt = pta.tile([128, N], f16, tag="pta", name=f"ptt{ib}")
            for k in range(8):
                nc.tensor.transpose(ptt[:, k * 128:(k + 1) * 128],
                                    adjm[:, ib * N + k * 128: ib * N + (k + 1) * 128],
                                    id16[:])
            dstv = adjT[:].rearrange("p (k b z) -> p k b z", k=8, b=2)[:, :, ib, :]
            nc.vector.tensor_copy(dstv, ptt[:].rearrange("p (k z) -> p k z", k=8))

        # ---------- xr_mod (deferred build, emitted during super 1) ----------
        xr_mod = consts.tile([128, 8 * 68], f16, tag="xrmod")

        def build_xr_mod():
            # slT[h, j] = sum_hc att_blk[hc,h]*xl[hc,j];  esl = exp(0.2*sl)
            psl = psg.tile([H, N], f32, tag="g", name="psl")
            for half in range(2):
                s = slice(half * 512, (half + 1) * 512)
                nc.tensor.matmul(psl[:, s], attb_r[:], xlT32[:, s],
                                 start=True, stop=True)
            eslT = work.tile([H, N], f32, tag="eslT", name="eslT")
            nc.scalar.activation(eslT[:], psl[:], Act.Exp, scale=0.2)
            esl_nat = work.tile([128, 8 * H], f32, tag="eslnat", name="esl_nat")
            gsc3 = psg.tile([128, N], f32, tag="g", name="gsc_esl")
            for k in range(8):
                nc.tensor.transpose(gsc3[:, k * 128:k * 128 + H],
                                    eslT[:, k * 128:(k + 1) * 128], id_t[0:H, 0:H])
                nc.tensor.transpose(gsc3[:, k * 128 + 16:k * 128 + 16 + HC],
                                    xrT[:, k * 128:(k + 1) * 128], id_t[0:HC, 0:HC])
            for k in range(8):
                nc.vector.tensor_copy(esl_nat[:, k * H:(k + 1) * H],
                                      gsc3[:, k * 128:k * 128 + H])
                zcols = xr_mod[:].rearrange("p (k h c) -> p k h c", k=8, h=H)[:, k, :, 16]
                nc.vector.tensor_copy(zcols, gsc3[:, k * 128:k * 128 + H])
                xcols = xr_mod[:].rearrange("p (k h c) -> p k h c", k=8, h=H)[:, k, :, 0:16]
                srcx = gsc3[:, k * 128 + 16:k * 128 + 16 + HC].rearrange(
                    "p (h c) -> p h c", h=H)
                rep = esl_nat[:, k * H:(k + 1) * H].rearrange(
                    "p (h one) -> p h one", one=1).broadcast_to([128, H, 16])
                nc.vector.tensor_tensor(xcols, srcx, rep, Alu.mult)

        # ---------- main streaming loop ----------
        st_t = [consts.tile([128, 8 * 512], f16, tag=f"stt{ib}", name=f"stt{ib}")
                for ib in range(2)]

        def aggregate(ib):
            out_f = outp.tile([128, HC], f32, tag="outf", name="outf")
            out_f2 = outp.tile([128, HC], f32, tag="outf2", name="outf2")
            for h in range(H):
                agg = pta.tile([128, 17], f32, tag="pta", name="agg")
                for k in range(8):
                    lhs = st_t[ib][:].rearrange("p (k s a h) -> p k s a h",
                                                k=8, s=4, h=H)[:, k, :, :, h]
                    rhs = xr_mod[:, k * 68 + h * 17: k * 68 + (h + 1) * 17]
                    nc.tensor.matmul(agg[:], lhs, rhs,
                                     start=(k == 0), stop=(k == 7))
                rz = work.tile([128, 1], f32, tag="rz", name="rz")
                nc.vector.reciprocal(rz[:], agg[:, 16:17])
                nc.vector.tensor_scalar(out_f[:, h * 16:(h + 1) * 16],
                                        agg[:, 0:16], rz[:, 0:1], None, Alu.mult)
            nc.vector.tensor_add(out_f2[:], out_f[:], biasb_t[:])
            dma(out[ib * 128:(ib + 1) * 128, :], out_f2[:])

        NF16 = NSUP - NSUP8
        rp16_i = 0
        rp8_i = 0
        for sup in range(NSUP):
            ib, s4 = sup // 4, sup % 4
            if sup == 1:
                build_xr_mod()
            pt_all = pta.tile([128, N], f16, tag="pta", name=f"pt{sup}")
            if sup < NF16:
                # ---- f16 path: 4 groups of 4 pairs ----
                for g in range(4):
                    gps = psg.tile([128, N], f32, tag="g", name="gps")
                    for q4 in range(4):
                        p = sup * 16 + g * 4 + q4
                        rp = rp_pool.tile([128, N], f16, tag="rp")
                        eng = RP16_ENGINES[rp16_i % len(RP16_ENGINES)]
                        rp16_i += 1
                        rp_issue(eng, rp[:], xl2T[:], xrp[:, p:p + 1])
                        for half in range(2):
                            s = slice(half * 512, (half + 1) * 512)
                            nc.tensor.matmul(gps[32 * q4:32 * q4 + 32, s],
                                             att2p_r[:], rp[:, s],
                                             start=True, stop=True,
                                             tile_position=(0, 32 * q4))
                    ssb = ssb_pool.tile([128, N], f16, tag="ssb")
                    nc.scalar.activation(ssb[:], gps[:], Act.Exp)
                    for k in range(8):
                        nc.tensor.transpose(
                            pt_all[:, k * 128 + g * 32:k * 128 + (g + 1) * 32],
                            ssb[:, k * 128:(k + 1) * 128], sel32[:])
            else:
                # ---- fp8 DoubleRow path: 2 tiles of 4 pair-pairs ----
                for t in range(2):
                    gps = psg.tile([128, N], f32, tag="g", name="gps8")
                    for q in range(4):
                        p0 = sup * 16 + t * 8 + 2 * q
                        rp8 = rp8_pool.tile([128, 2, N], f8e4, tag="rp8")
                        for pb in range(2):
                            eng = RP8_ENGINES[rp8_i % len(RP8_ENGINES)]
                            rp8_i += 1
                            rp_issue(eng, rp8[:, pb, :], xl2T8[:],
                                     xrp8[:, p0 + pb:p0 + pb + 1])
                        for half in range(2):
                            s = slice(half * 512, (half + 1) * 512)
                            nc.tensor.matmul(gps[32 * q:32 * q + 32, s],
                                             att8[:], rp8[:, :, s],
                                             start=True, stop=True,
                                             tile_position=(0, 32 * q),
                                             perf_mode=mybir.MatmulPerfMode.DoubleRow)
                    ssb = ssb_pool.tile([128, N], f16, tag="ssb")
                    nc.scalar.activation(ssb[:], gps[:], Act.Exp)
                    for k in range(8):
                        nc.tensor.transpose(
                            pt_all[:, k * 128 + t * 64:k * 128 + (t + 1) * 64],
                            ssb[:, k * 128:(k + 1) * 128], sel64[:])
            # ---- mask + write st ----
            # Pool cannot read PSUM: either mask directly on DVE from pt_all,
            # or bounce pt_all to SBUF once and mask on Pool.
            if MASK_ENGINE == "p":
                ptc = rp_pool.tile([128, N], f16, tag="ptc", name=f"ptc{sup}")
                if PTCOPY_ENGINE == "a":
                    nc.scalar.activation(ptc[:], pt_all[:], Act.Identity)
                else:
                    nc.vector.tensor_copy(ptc[:], pt_all[:])
                src_t = ptc
            else:
                src_t = pt_all
            for k in range(8):
                msk = adjT[:, k * 256 + ib * 128 + s4 * 32:
                           k * 256 + ib * 128 + s4 * 32 + 32]
                mskr = msk.rearrange("p (a one) -> p a one", one=1).broadcast_to(
                    [128, 32, H])
                dstv = st_t[ib][:, k * 512 + s4 * 128: k * 512 + (s4 + 1) * 128]
                dstv = dstv.rearrange("p (a h) -> p a h", h=H)
                ptv = src_t[:, k * 128:(k + 1) * 128].rearrange(
                    "p (a h) -> p a h", h=H)
                if MASK_ENGINE == "p":
                    nc.gpsimd.tensor_tensor(dstv, ptv, mskr, Alu.mult)
                else:
                    nc.vector.tensor_tensor(dstv, ptv, mskr, Alu.mult)

        aggregate(0)
        aggregate(1)


def _get_program():
    if "nc" not in _CACHE:
        _CACHE["nc"] = _build_program()
    return _CACHE["nc"]


def kernel(x, adj, W_l, b_l, W_r, b_r, att, bias):
    global LAST_RESULTS
    from concourse.bass_utils import run_bass_kernel_spmd

    x = np.ascontiguousarray(np.asarray(x, dtype=np.float32))
    adj = np.ascontiguousarray(np.asarray(adj, dtype=np.float32))
    W_l = np.asarray(W_l, dtype=np.float32)
    b_l = np.asarray(b_l, dtype=np.float32)
    W_r = np.asarray(W_r, dtype=np.float32)
    b_r = np.asarray(b_r, dtype=np.float32)
    att = np.asarray(att, dtype=np.float32)
    bias = np.asarray(bias, dtype=np.float32)

    # host-side constant prep
    att2p = np.zeros((F, 32), np.float32)        # [0.8 * att2 | 0]
    for d in range(2):
        for h in range(H):
            att2p[d * HC + h * C:(d * HC + (h + 1) * C), d * H + h] = 0.8 * att[h]
    attb = np.zeros((HC, H), np.float32)         # att_blk
    for h in range(H):
        attb[h * C:(h + 1) * C, h] = att[h]
    identity = np.eye(128, dtype=np.float32)
    biasb = np.broadcast_to(bias, (128, HC)).copy()
    blp = b_l.reshape(HC, 1).copy()
    brp = b_r.reshape(HC, 1).copy()

    # fp8 path: on-grid quantized att (device f32->f8e4 convert is exact)
    # and row scales r = att/q8(att) folded into xl/xr (relu(r*z) = r*relu(z)).
    a8flat = _q8(0.8 * att.reshape(-1))          # [HC] on e4m3 grid
    r1 = np.where(a8flat != 0.0, (0.8 * att.reshape(-1)) / np.where(a8flat == 0, 1, a8flat), 1.0)
    att8w = np.zeros((128, 64), np.float32)      # [p, t*32+m]
    for d in range(2):
        for h in range(H):
            for t in range(2):
                m = 8 * t + d * H + h
                att8w[d * HC + h * C:(d * HC + (h + 1) * C), t * 32 + m] = \
                    a8flat[h * C:(h + 1) * C]
    r2v = np.concatenate([r1, r1]).reshape(128, 1).astype(np.float32)

    # selection matrices: sel32 picks rows 32q+r (r<8) -> col q*8+r
    # sel64 picks rows 32q+r (r<16) -> col q*16+r
    selw = np.zeros((128, 96), np.float32)
    for q in range(4):
        for r in range(8):
            selw[32 * q + r, q * 8 + r] = 1.0
        for r in range(16):
            selw[32 * q + r, 32 + q * 16 + r] = 1.0

    in_maps = []
    for core in range(NCORES):
        b, blk = core // 4, core % 4
        i0 = blk * NI
        adjs = adj[b, i0:i0 + NI, :].copy()
        adjs[np.arange(NI), i0 + np.arange(NI)] = 1.0   # self loops
        adjs = adjs.astype(np.float16)
        in_maps.append({
            "xb": x[b], "xis": x[b, i0:i0 + NI].copy(), "adjs": adjs,
            "wl": W_l, "wr": W_r, "blp": blp, "brp": brp,
            "att2p": att2p, "attb": attb, "ident": identity, "biasb": biasb,
            "att8w": att8w, "r2": r2v, "selw": selw,
        })

    nc = _get_program()
    res = run_bass_kernel_spmd(nc, in_maps, core_ids=list(range(NCORES)))
    LAST_RESULTS = res
    outp = np.zeros((B, N, HC), np.float32)
    for core in range(NCORES):
        b, blk = core // 4, core % 4
        outp[b, blk * NI:(blk + 1) * NI, :] = res.results[core]["out"]
    return outp


# revision 15
# speedup vs baseline: 1.2685x; 1.0512x over previous
# DenseGATv2Conv Trainium2 kernel (v2).
#
# Math (per batch b):
#   xl = x @ W_l + b_l ; xr = x @ W_r + b_r            [N, H*C]
#   alpha[i,j,h] = sum_c att[h,c] * leaky_relu(xl[j,hc] + xr[i,hc], 0.2)
#   S = softmax_j(alpha masked by adj(+self loops))
#   out[i,hc] = sum_j S[i,j,h] * xr[j,hc] + bias
#
# Identity used on device:
#   leaky_relu(z) = 0.2*z + 0.8*relu(z)
#   alpha[i,j,h] = 0.2*sl[j,h] + 0.2*sr[i,h] + 0.8*sum_c att[h,c]*relu(...)
# exp(0.2*sr[i,h]) cancels in softmax; exp(0.2*sl[j,h]) is folded into the
# aggregation operand (xr_mod).  Per pair of destination rows (i0,i1) the
# relu tensor rp[128=2x64 hc, N j] feeds a PE contraction with att.
#
# v2 structure (vs baseline):
#  - scores for the last NSUP8 supers use fp8e4 + DoubleRow matmuls packing
#    TWO pairs per matmul (0.5 cy/row): 4x tensor-engine rate, and halves
#    the Act-engine exp count (16 useful rows per 32-row PSUM block).
#    att quantization error is cancelled by pre-scaling xl/xr rows with
#    r = att/q8(att) (relu is positively homogeneous).
#  - partition compaction of the sparse score blocks is fused into the PE
#    transpose by using a selection matrix instead of the identity
#    (out = ssb^T @ sel), eliminating all 128 SBUF->SBUF gather DMAs of the
#    baseline (HWDGE was 81% busy).
#  - per-super transposes write one PSUM tile (pt_all); masking by adjT
#    runs on the otherwise-idle Pool engine and writes st directly.
#  - relu ops are spread across DVE/Act/Pool to balance engine busy time.
#
# Sharding: 8 cores = (batch b in 0..1) x (4 blocks of 256 destination rows).

import numpy as np

B, N, F, H, C = 2, 1024, 128, 4, 16
HC = H * C
NCORES = 8
NI = 256          # destination rows per core
NPAIR = NI // 2   # 128 pairs of destination rows
NSUP = 8          # supers of 16 pairs (32 dest rows) each
NSUP8 = 0         # how many (trailing) supers use the fp8 DoubleRow path

# engine assignment for the fp8 relu ops (cycled): d=DVE, a=Act, p=Pool
RP8_ENGINES = "ddapdap"
# engine assignment for f16 relu ops
RP16_ENGINES = "d"
# mask op engine: "p" = Pool (needs a PSUM->SBUF bounce copy), "d" = DVE direct
MASK_ENGINE = "p"
# engine for the pt_all PSUM->SBUF bounce copy: "d"=DVE, "a"=Act
PTCOPY_ENGINE = "d"

_CACHE = {}
LAST_RESULTS = None


def _q8(v):
    """Round to float8e4m3 grid (RNE), pure numpy. No inf/nan handling."""
    v = np.clip(np.asarray(v, np.float64), -448.0, 448.0)
    m, e = np.frexp(v)  # v = m * 2**e, |m| in [0.5, 1)
    normal = np.abs(v) >= 2.0 ** -6
    # normal: round to 4-bit significand at scale 2**(e-4)
    qn = np.round(v * 2.0 ** (4 - e)) * 2.0 ** (e - 4.0)
    # subnormal: step 2**-9
    qs = np.round(v * 2.0 ** 9) * 2.0 ** -9
    return np.where(normal, qn, qs).astype(np.float32)


def _build_program(debug=False):
    import concourse.bass as bass
    import concourse.mybir as mybir
    import concourse.tile as tile
    from concourse import bacc

    f32 = mybir.dt.float32
    f32r = mybir.dt.float32r
    f16 = mybir.dt.float16
    f8e4 = mybir.dt.float8e4
    Alu = mybir.AluOpType
    Act = mybir.ActivationFunctionType

    nc = bacc.Bacc(
        "TRN2",
        target_bir_lowering=False,
        debug=False,
        enable_asserts=False,
        num_devices=NCORES,
    )

    # ---- DRAM I/O ----
    xb = nc.dram_tensor("xb", [N, F], f32, kind="ExternalInput").ap()
    xis = nc.dram_tensor("xis", [NI, F], f32, kind="ExternalInput").ap()
    adjs = nc.dram_tensor("adjs", [NI, N], f16, kind="ExternalInput").ap()
    wl = nc.dram_tensor("wl", [F, HC], f32, kind="ExternalInput").ap()
    wr = nc.dram_tensor("wr", [F, HC], f32, kind="ExternalInput").ap()
    blp = nc.dram_tensor("blp", [HC, 1], f32, kind="ExternalInput").ap()
    brp = nc.dram_tensor("brp", [HC, 1], f32, kind="ExternalInput").ap()
    att2p = nc.dram_tensor("att2p", [F, 32], f32, kind="ExternalInput").ap()
    attb = nc.dram_tensor("attb", [HC, H], f32, kind="ExternalInput").ap()
    ident = nc.dram_tensor("ident", [128, 128], f32, kind="ExternalInput").ap()
    biasb = nc.dram_tensor("biasb", [128, HC], f32, kind="ExternalInput").ap()
    att8w = nc.dram_tensor("att8w", [128, 64], f32, kind="ExternalInput").ap()
    r2 = nc.dram_tensor("r2", [128, 1], f32, kind="ExternalInput").ap()
    selw = nc.dram_tensor("selw", [128, 96], f32, kind="ExternalInput").ap()
    out = nc.dram_tensor("out", [NI, HC], f32, kind="ExternalOutput").ap()

    with tile.TileContext(nc) as tc:
        _body(tc, nc, mybir, bass, f32, f32r, f16, f8e4, Alu, Act,
              xb, xis, adjs, wl, wr, blp, brp, att2p, attb, ident, biasb,
              att8w, r2, selw, out)

    nc.compile()
    return nc


def _body(tc, nc, mybir, bass, f32, f32r, f16, f8e4, Alu, Act,
          xb, xis, adjs, wl, wr, blp, brp, att2p, attb, ident, biasb,
          att8w, r2, selw, out):
    from contextlib import ExitStack
    ctx = ExitStack()
    with ctx:
        consts = ctx.enter_context(tc.tile_pool(name="consts", bufs=1))
        work = ctx.enter_context(tc.tile_pool(name="work", bufs=1))
        rp_pool = ctx.enter_context(tc.tile_pool(name="rp", bufs=6))
        rp8_pool = ctx.enter_context(tc.tile_pool(name="rp8", bufs=4))
        ssb_pool = ctx.enter_context(tc.tile_pool(name="ssb", bufs=3))
        outp = ctx.enter_context(tc.tile_pool(name="outp", bufs=2))
        psg = ctx.enter_context(tc.tile_pool(name="psg", bufs=3, space="PSUM"))
        pta = ctx.enter_context(tc.tile_pool(name="pta", bufs=2, space="PSUM"))

        dma = nc.sync.dma_start

        def rp_issue(eng, dst, src, bias_col):
            if eng == "d":
                nc.vector.tensor_scalar(dst, src, bias_col, 0.0, Alu.add, Alu.max)
            elif eng == "a":
                nc.scalar.activation(dst, src, Act.Relu, bias=bias_col, scale=1.0)
            else:
                nc.gpsimd.tensor_scalar(dst, src, bias_col, 0.0, Alu.add, Alu.max)

        # ---------- load constants ----------
        # HWDGE (SP queue) carries only the startup-critical loads, in
        # dependency order; everything else rides the Pool-engine SWDGE so the
        # first PE transpose can start ~2us in.
        wl_t = consts.tile([F, HC], f32, tag="wl")
        wr_t = consts.tile([F, HC], f32, tag="wr")
        blp_t = consts.tile([HC, 1], f32, tag="blp")
        brp_t = consts.tile([HC, 1], f32, tag="brp")
        att2p_t = consts.tile([F, 32], f32, tag="att2p")
        att2p_r = consts.tile([F, 32], f16, tag="att2pr")
        attb_t = consts.tile([HC, H], f32, tag="attb")
        wl_r = consts.tile([F, HC], f32r, tag="wlr")
        wr_r = consts.tile([F, HC], f32r, tag="wrr")
        attb_r = consts.tile([HC, H], f32r, tag="attbr")
        id_t = consts.tile([128, 128], f32, tag="ident")
        biasb_t = consts.tile([128, HC], f32, tag="biasb")
        selw_t = consts.tile([128, 96], f32, tag="selw")
        sel32 = consts.tile([128, 32], f16, tag="sel32")
        sel64 = consts.tile([128, 64], f16, tag="sel64")
        id16 = consts.tile([128, 128], f16, tag="id16")
        xin = consts.tile([128, 8 * F], f32, tag="xin")       # x[b] tiles
        xis_t = consts.tile([128, 2 * F], f32, tag="xis")
        adjm = consts.tile([128, 2 * N], f16, tag="adjm")     # [i128][ib*N + j]

        # critical-path loads on HWDGE, batched into single multi-dim DMAs
        dma(id_t[:], ident)
        for k in range(2):
            dma(xis_t[:, k * F:(k + 1) * F], xis[k * 128:(k + 1) * 128, :])
        for k in range(8):
            dma(xin[:, k * F:(k + 1) * F], xb[k * 128:(k + 1) * 128, :])
        dma(wr_t[:], wr)
        dma(wl_t[:], wl)
        dma(brp_t[:], brp)
        dma(blp_t[:], blp)
        dma(att2p_t[:], att2p)
        # the rest on SWDGE (Pool engine, idle early)
        nc.gpsimd.dma_start(selw_t[:], selw)
        for ib in range(2):
            nc.gpsimd.dma_start(adjm[:, ib * N:(ib + 1) * N],
                                adjs[ib * 128:(ib + 1) * 128, :])
        nc.gpsimd.dma_start(attb_t[:], attb)
        nc.gpsimd.dma_start(biasb_t[:], biasb)
        if NSUP8 > 0:
            att8w_t = consts.tile([128, 64], f32, tag="att8w")
            att8 = consts.tile([128, 2, 32], f8e4, tag="att8")
            r2_t = consts.tile([128, 1], f32, tag="r2")
            nc.gpsimd.dma_start(att8w_t[:], att8w)
            nc.gpsimd.dma_start(r2_t[:], r2)
            nc.gpsimd.tensor_copy(att8[:].rearrange("p t m -> p (t m)"), att8w_t[:])
        nc.gpsimd.tensor_copy(sel32[:], selw_t[:, 0:32])
        if NSUP8 > 0:
            nc.gpsimd.tensor_copy(sel64[:], selw_t[:, 32:96])

        # ---------- xisT/xT via PE transpose (batched through psum tiles) ----
        # xis chain first: it feeds xrp which gates the first relu op.
        xisT = consts.tile([F, NI], f32r, tag="xisT")
        gsc2 = psg.tile([128, N], f32, tag="g", name="gsc_xisT")
        for k in range(2):
            nc.tensor.transpose(gsc2[:, k * 128:(k + 1) * 128],
                                xis_t[:, k * F:(k + 1) * F], id_t[:])
        nc.vector.tensor_copy(xisT[:], gsc2[:, 0:NI])
        nc.vector.tensor_copy(wr_r[:], wr_t[:])
        nc.vector.tensor_copy(wl_r[:], wl_t[:])
        xT = consts.tile([F, N], f32r, tag="xT")              # [f, node]
        gsc = psg.tile([128, N], f32, tag="g", name="gsc_xT")
        for k in range(8):
            nc.tensor.transpose(gsc[:, k * 128:(k + 1) * 128],
                                xin[:, k * F:(k + 1) * F], id_t[:])
        nc.vector.tensor_copy(xT[:], gsc[:])

        # ---------- projections ----------
        xl2T = consts.tile([128, N], f16, tag="xl2T")         # rows 0:64 == 64:128
        xlT32 = consts.tile([HC, N], f32r, tag="xlT32")
        xrT = consts.tile([HC, N], f32, tag="xrT")
        xrsT = consts.tile([HC, NI], f32, tag="xrsT")
        pj3 = psg.tile([HC, NI], f32, tag="g", name="pj3")
        nc.tensor.matmul(pj3[:], wr_r[:], xisT[:], start=True, stop=True)
        nc.scalar.activation(xrsT[:], pj3[:], Act.Identity,
                             bias=brp_t[:, 0:1], scale=1.0)
        pj = psg.tile([HC, N], f32, tag="g", name="pj")
        for half in range(2):
            s = slice(half * 512, (half + 1) * 512)
            nc.tensor.matmul(pj[:, s], wl_r[:], xT[:, s], start=True, stop=True)
        nc.scalar.activation(xl2T[0:HC, :], pj[:], Act.Identity,
                             bias=blp_t[:, 0:1], scale=1.0)
        nc.vector.tensor_copy(xl2T[HC:128, :], xl2T[0:HC, :])
        nc.scalar.activation(xlT32[:], pj[:], Act.Identity,
                             bias=blp_t[:, 0:1], scale=1.0)
        pj2 = psg.tile([HC, N], f32, tag="g", name="pj2")
        for half in range(2):
            s = slice(half * 512, (half + 1) * 512)
            nc.tensor.matmul(pj2[:, s], wr_r[:], xT[:, s], start=True, stop=True)
        nc.scalar.activation(xrT[:], pj2[:], Act.Identity,
                             bias=brp_t[:, 0:1], scale=1.0)
        nc.vector.tensor_copy(att2p_r[:], att2p_t[:])

        # ---------- xrp: per-pair bias columns [xr[i0+2p] ; xr[i0+2p+1]] ----------
        xrp = consts.tile([128, NPAIR], f32, tag="xrp")
        ev = xrsT[:].rearrange("p (a two) -> p a two", two=2)
        nc.vector.tensor_copy(xrp[0:HC, :], ev[:, :, 0])
        nc.vector.tensor_copy(xrp[HC:128, :], ev[:, :, 1])

        # fp8 path: r-scaled copies (cancels att quantization error)
        if NSUP8 > 0:
            xl2T8 = consts.tile([128, N], f16, tag="xl2T8")
            xrp8 = consts.tile([128, NPAIR], f32, tag="xrp8")
            nc.vector.tensor_scalar(xl2T8[:], xl2T[:], r2_t[:, 0:1], None, Alu.mult)
            nc.vector.tensor_scalar(xrp8[:], xrp[:], r2_t[:, 0:1], None, Alu.mult)

        # ---------- adjT (deferred; emitted between sup0 compute and masks) --
        adjT = consts.tile([128, 2 * N], f16, tag="adjT")

        def build_adjT():
            nc.vector.tensor_copy(id16[:], id_t[:])
            for ib in range(2):
                ptt = pta.tile([128, N], f16, tag="pta", name=f"ptt{ib}")
                for k in range(8):
                    nc.tensor.transpose(ptt[:, k * 128:(k + 1) * 128],
                                        adjm[:, ib * N + k * 128: ib * N + (k + 1) * 128],
                                        id16[:])
                dstv = adjT[:].rearrange("p (k b z) -> p k b z", k=8, b=2)[:, :, ib, :]
                nc.vector.tensor_copy(dstv, ptt[:].rearrange("p (k z) -> p k z", k=8))

        # ---------- xr_mod (deferred build, emitted during super 1) ----------
        xr_mod = consts.tile([128, 8 * 68], f16, tag="xrmod")

        def build_xr_mod():
            # slT[h, j] = sum_hc att_blk[hc,h]*xl[hc,j];  esl = exp(0.2*sl)
            nc.vector.tensor_copy(attb_r[:], attb_t[:])
            psl = psg.tile([H, N], f32, tag="g", name="psl")
            for half in range(2):
                s = slice(half * 512, (half + 1) * 512)
                nc.tensor.matmul(psl[:, s], attb_r[:], xlT32[:, s],
                                 start=True, stop=True)
            eslT = work.tile([H, N], f32, tag="eslT", name="eslT")
            nc.scalar.activation(eslT[:], psl[:], Act.Exp, scale=0.2)
            esl_nat = work.tile([128, 8 * H], f32, tag="eslnat", name="esl_nat")
            gsc3 = psg.tile([128, N], f32, tag="g", name="gsc_esl")
            for k in range(8):
                nc.tensor.transpose(gsc3[:, k * 128:k * 128 + H],
                                    eslT[:, k * 128:(k + 1) * 128], id_t[0:H, 0:H])
                nc.tensor.transpose(gsc3[:, k * 128 + 16:k * 128 + 16 + HC],
                                    xrT[:, k * 128:(k + 1) * 128], id_t[0:HC, 0:HC])
            for k in range(8):
                nc.vector.tensor_copy(esl_nat[:, k * H:(k + 1) * H],
                                      gsc3[:, k * 128:k * 128 + H])
                zcols = xr_mod[:].rearrange("p (k h c) -> p k h c", k=8, h=H)[:, k, :, 16]
                nc.vector.tensor_copy(zcols, gsc3[:, k * 128:k * 128 + H])
                xcols = xr_mod[:].rearrange("p (k h c) -> p k h c", k=8, h=H)[:, k, :, 0:16]
                srcx = gsc3[:, k * 128 + 16:k * 128 + 16 + HC].rearrange(
                    "p (h c) -> p h c", h=H)
                rep = esl_nat[:, k * H:(k + 1) * H].rearrange(
                    "p (h one) -> p h one", one=1).broadcast_to([128, H, 16])
                nc.vector.tensor_tensor(xcols, srcx, rep, Alu.mult)

        # ---------- main streaming loop ----------
        st_t = [consts.tile([128, 8 * 512], f16, tag=f"stt{ib}", name=f"stt{ib}")
                for ib in range(2)]

        def aggregate(ib):
            out_f = outp.tile([128, HC], f32, tag="outf", name="outf")
            out_f2 = outp.tile([128, HC], f32, tag="outf2", name="outf2")
            for h in range(H):
                agg = pta.tile([128, 17], f32, tag="pta", name="agg")
                for k in range(8):
                    lhs = st_t[ib][:].rearrange("p (k s a h) -> p k s a h",
                                                k=8, s=4, h=H)[:, k, :, :, h]
                    rhs = xr_mod[:, k * 68 + h * 17: k * 68 + (h + 1) * 17]
                    nc.tensor.matmul(agg[:], lhs, rhs,
                                     start=(k == 0), stop=(k == 7))
                rz = work.tile([128, 1], f32, tag="rz", name="rz")
                nc.vector.reciprocal(rz[:], agg[:, 16:17])
                nc.vector.tensor_scalar(out_f[:, h * 16:(h + 1) * 16],
                                        agg[:, 0:16], rz[:, 0:1], None, Alu.mult)
            nc.vector.tensor_add(out_f2[:], out_f[:], biasb_t[:])
            dma(out[ib * 128:(ib + 1) * 128, :], out_f2[:])

        NF16 = NSUP - NSUP8
        rp16_i = 0
        rp8_i = 0
        for sup in range(NSUP):
            ib, s4 = sup // 4, sup % 4
            if sup == 1:
                build_xr_mod()
            pt_all = pta.tile([128, N], f16, tag="pta", name=f"pt{sup}")
            if sup < NF16:
                # ---- f16 path: 4 groups of 4 pairs ----
                for g in range(4):
                    gps = psg.tile([128, N], f32, tag="g", name="gps")
                    for q4 in range(4):
                        p = sup * 16 + g * 4 + q4
                        rp = rp_pool.tile([128, N], f16, tag="rp")
                        eng = RP16_ENGINES[rp16_i % len(RP16_ENGINES)]
                        rp16_i += 1
                        rp_issue(eng, rp[:], xl2T[:], xrp[:, p:p + 1])
                        for half in range(2):
                            s = slice(half * 512, (half + 1) * 512)
                            nc.tensor.matmul(gps[32 * q4:32 * q4 + 32, s],
                                             att2p_r[:], rp[:, s],
                                             start=True, stop=True,
                                             tile_position=(0, 32 * q4))
                    ssb = ssb_pool.tile([128, N], f16, tag="ssb")
                    nc.scalar.activation(ssb[:], gps[:], Act.Exp)
                    for k in range(8):
                        nc.tensor.transpose(
                            pt_all[:, k * 128 + g * 32:k * 128 + (g + 1) * 32],
                            ssb[:, k * 128:(k + 1) * 128], sel32[:])
            else:
                # ---- fp8 DoubleRow path: 2 tiles of 4 pair-pairs ----
                for t in range(2):
                    gps = psg.tile([128, N], f32, tag="g", name="gps8")
                    for q in range(4):
                        p0 = sup * 16 + t * 8 + 2 * q
                        rp8 = rp8_pool.tile([128, 2, N], f8e4, tag="rp8")
                        for pb in range(2):
                            eng = RP8_ENGINES[rp8_i % len(RP8_ENGINES)]
                            rp8_i += 1
                            rp_issue(eng, rp8[:, pb, :], xl2T8[:],
                                     xrp8[:, p0 + pb:p0 + pb + 1])
                        for half in range(2):
                            s = slice(half * 512, (half + 1) * 512)
                            nc.tensor.matmul(gps[32 * q:32 * q + 32, s],
                                             att8[:], rp8[:, :, s],
                                             start=True, stop=True,
                                             tile_position=(0, 32 * q),
                                             perf_mode=mybir.MatmulPerfMode.DoubleRow)
                    ssb = ssb_pool.tile([128, N], f16, tag="ssb")
                    nc.scalar.activation(ssb[:], gps[:], Act.Exp)
                    for k in range(8):
                        nc.tensor.transpose(
                            pt_all[:, k * 128 + t * 64:k * 128 + (t + 1) * 64],
                            ssb[:, k * 128:(k + 1) * 128], sel64[:])
            if sup == 0:
                build_adjT()
            # ---- mask + write st ----
            # Pool cannot read PSUM: either mask directly on DVE from pt_all,
            # or bounce pt_all to SBUF once and mask on Pool. The last super's
            # masks go direct on DVE to shorten the tail.
            eng_m = MASK_ENGINE if sup < NSUP - 1 else "d"
            if eng_m == "p":
                ptc = rp_pool.tile([128, N], f16, tag="ptc", name=f"ptc{sup}")
                if PTCOPY_ENGINE == "a":
                    nc.scalar.activation(ptc[:], pt_all[:], Act.Identity)
                else:
                    nc.vector.tensor_copy(ptc[:], pt_all[:])
                src_t = ptc
            else:
                src_t = pt_all
            for k in range(8):
                msk = adjT[:, k * 256 + ib * 128 + s4 * 32:
                           k * 256 + ib * 128 + s4 * 32 + 32]
                mskr = msk.rearrange("p (a one) -> p a one", one=1).broadcast_to(
                    [128, 32, H])
                dstv = st_t[ib][:, k * 512 + s4 * 128: k * 512 + (s4 + 1) * 128]
                dstv = dstv.rearrange("p (a h) -> p a h", h=H)
                ptv = src_t[:, k * 128:(k + 1) * 128].rearrange(
                    "p (a h) -> p a h", h=H)
                if eng_m == "p":
                    nc.gpsimd.tensor_tensor(dstv, ptv, mskr, Alu.mult)
                else:
                    nc.vector.tensor_tensor(dstv, ptv, mskr, Alu.mult)
            if sup == 3:
                aggregate(0)   # st[0] complete; overlap with sups 4..7

        aggregate(1)


def _get_program():
    if "nc" not in _CACHE:
        _CACHE["nc"] = _build_program()
    return _CACHE["nc"]


def kernel(x, adj, W_l, b_l, W_r, b_r, att, bias):
    global LAST_RESULTS
    from concourse.bass_utils import run_bass_kernel_spmd

    x = np.ascontiguousarray(np.asarray(x, dtype=np.float32))
    adj = np.ascontiguousarray(np.asarray(adj, dtype=np.float32))
    W_l = np.asarray(W_l, dtype=np.float32)
    b_l = np.asarray(b_l, dtype=np.float32)
    W_r = np.asarray(W_r, dtype=np.float32)
    b_r = np.asarray(b_r, dtype=np.float32)
    att = np.asarray(att, dtype=np.float32)
    bias = np.asarray(bias, dtype=np.float32)

    # host-side constant prep
    att2p = np.zeros((F, 32), np.float32)        # [0.8 * att2 | 0]
    for d in range(2):
        for h in range(H):
            att2p[d * HC + h * C:(d * HC + (h + 1) * C), d * H + h] = 0.8 * att[h]
    attb = np.zeros((HC, H), np.float32)         # att_blk
    for h in range(H):
        attb[h * C:(h + 1) * C, h] = att[h]
    identity = np.eye(128, dtype=np.float32)
    biasb = np.broadcast_to(bias, (128, HC)).copy()
    blp = b_l.reshape(HC, 1).copy()
    brp = b_r.reshape(HC, 1).copy()

    # fp8 path: on-grid quantized att (device f32->f8e4 convert is exact)
    # and row scales r = att/q8(att) folded into xl/xr (relu(r*z) = r*relu(z)).
    a8flat = _q8(0.8 * att.reshape(-1))          # [HC] on e4m3 grid
    r1 = np.where(a8flat != 0.0, (0.8 * att.reshape(-1)) / np.where(a8flat == 0, 1, a8flat), 1.0)
    att8w = np.zeros((128, 64), np.float32)      # [p, t*32+m]
    for d in range(2):
        for h in range(H):
            for t in range(2):
                m = 8 * t + d * H + h
                att8w[d * HC + h * C:(d * HC + (h + 1) * C), t * 32 + m] = \
                    a8flat[h * C:(h + 1) * C]
    r2v = np.concatenate([r1, r1]).reshape(128, 1).astype(np.float32)

    # selection matrices: sel32 picks rows 32q+r (r<8) -> col q*8+r
    # sel64 picks rows 32q+r (r<16) -> col q*16+r
    selw = np.zeros((128, 96), np.float32)
    for q in range(4):
        for r in range(8):
            selw[32 * q + r, q * 8 + r] = 1.0
        for r in range(16):
            selw[32 * q + r, 32 + q * 16 + r] = 1.0

    in_maps = []
    for core in range(NCORES):
        b, blk = core // 4, core % 4
        i0 = blk * NI
        adjs = adj[b, i0:i0 + NI, :].copy()
        adjs[np.arange(NI), i0 + np.arange(NI)] = 1.0   # self loops
        adjs = adjs.astype(np.float16)
        in_maps.append({
            "xb": x[b], "xis": x[b, i0:i0 + NI].copy(), "adjs": adjs,
            "wl": W_l, "wr": W_r, "blp": blp, "brp": brp,
            "att2p": att2p, "attb": attb, "ident": identity, "biasb": biasb,
            "att8w": att8w, "r2": r2v, "selw": selw,
        })

    nc = _get_program()
    res = run_bass_kernel_spmd(nc, in_maps, core_ids=list(range(NCORES)))
    LAST_RESULTS = res
    outp = np.zeros((B, N, HC), np.float32)
    for core in range(NCORES):
        b, blk = core // 4, core % 4
        outp[b, blk * NI:(blk + 1) * NI, :] = res.results[core]["out"]
    return outp
